# revision 38
# baseline (speedup 1.0000x reference)
"""Adaptive 5x5 per-pixel convolution on 8 TRN2 NeuronCores.

out[b,c,h,w] = sum_{i,j} x[b,c,h+i,w+j] * kernel[b,i*5+j,h,w]

Sharding: data-parallel over batch B=8 -> 1 batch per core.
Per-core: x [64, 260, 260], kernel [25, 256, 256] -> out [64, 256, 256].

Device layout: partitions = h-rows, free = (channel-group, w). Rows 0..247 in
two 124-row blocks over 8-channel groups; rows 248..255 "tail-packed" as
(channel, row) pairs on partitions (10 channels x 12 x-rows = 120 partitions).

Per (block, channel-group, tap-row i): two bf16 DVE tensor_tensor multiplies
(even-j taps from x, odd-j taps from a one-column-shifted copy, keeping the
4B alignment the DVE 2x mode needs) write all five tap products into one
product tile. TensorE matmuls with a shifted-identity stationary S_i undo the
tap row shift and accumulate all 25 taps into PSUM ([124, 512] = 2 channels
per matmul). ScalarE copies PSUM->SBUF (cast to bf16); DMA stores; host upcasts to f32.

Engine APs must start at 32-aligned partitions, so row shifts live in the
kernel-tile DMA (any partition base) + the stationary, never in a compute read.

Host-side: cast to bf16 and relayout x -> [h, c, w], kernel -> [i, h, j, w],
out <- [h, c, w] so DMA moves >=2KB contiguous runs per partition.
"""

import sys
from contextlib import ExitStack

import ml_dtypes
import numpy as np

sys.path.insert(0, "/opt/trn_rl_repo")

from concourse import bacc, bass, tile  # noqa: E402
from concourse import mybir  # noqa: E402
from concourse.bass_utils import run_bass_kernel_spmd  # noqa: E402

F32 = mybir.dt.float32
BF16 = mybir.dt.bfloat16
BF16_NP = ml_dtypes.bfloat16

C, HP, WP = 64, 260, 260
KK, H, W = 25, 256, 256
K = 5

BLOCKS = [0, 124]
NH = 124  # out rows per main block
XR = 128  # x rows per main tile
CG = 8  # channels per x/product tile
# product j-slot order: evens {0,2,4} then odds {1,3}
SLOT_OF_J = {0: 0, 2: 1, 4: 2, 1: 3, 3: 4}

# tail: out rows 248..255 from x rows 248..259
TH0, TXR, TNH = 248, 12, 8
TCG = 10  # channels per tail group
TGROUPS = [(0, 10), (10, 10), (20, 10), (30, 10), (40, 10), (50, 10), (60, 4)]
TP = TCG * TXR  # 120 tail partitions
TQ = TCG * TNH  # 80 tail psum partitions

_CACHE = {}


def _ap(t, off, dims):
    return bass.AP(t[:].tensor, off, dims)


def _build_nc():
    nc = bacc.Bacc(
        "TRN2", target_bir_lowering=False, debug=False, num_devices=8
    )
    x = nc.dram_tensor("x", [HP, C, WP], BF16, kind="ExternalInput").ap()
    k = nc.dram_tensor("k", [K, H, K, W], BF16, kind="ExternalInput").ap()
    s = nc.dram_tensor("s", [XR, K * XR], BF16, kind="ExternalInput").ap()
    st = nc.dram_tensor("st", [TP, K * TQ], BF16, kind="ExternalInput").ap()
    out = nc.dram_tensor("out", [H, C, W], BF16, kind="ExternalOutput").ap()

    with tile.TileContext(nc) as tc, ExitStack() as ctx:
        spool = ctx.enter_context(tc.tile_pool(name="spool", bufs=1))
        kpool = ctx.enter_context(tc.tile_pool(name="kpool", bufs=2))
        ktpool = ctx.enter_context(tc.tile_pool(name="ktpool", bufs=1))
        xpool = ctx.enter_context(tc.tile_pool(name="xpool", bufs=5))
        ppool = ctx.enter_context(tc.tile_pool(name="ppool", bufs=4))
        opool = ctx.enter_context(tc.tile_pool(name="opool", bufs=4))
        mmpool = ctx.enter_context(tc.tile_pool(name="mm", bufs=2, space="PSUM"))

        stile = spool.tile([XR, K * XR], BF16)
        sttile = spool.tile([TP, K * TQ], BF16)
        sdone = []

        # ---------------- tail prep (emitted early, prefetched) ----------
        ktt = []

        def emit_tail_prep(i):
            # k tail tile: [128, 5j*256]; partition (c*12+r) holds k row 248+r-i
            kt = ktpool.tile([128, K * W], BF16, tag=f"kt{i}", name=f"ktt{i}")
            nc.gpsimd.memset(kt[:], 0.0)
            for c in range(TCG):
                nc.gpsimd.dma_start(
                    kt[c * TXR + i : c * TXR + i + TNH, :].rearrange(
                        "p (j w) -> p j w", j=K
                    ),
                    k[i, TH0 : TH0 + TNH, :, :],
                )
            ktt.append(kt)

        def emit_tail_pair(pair):
            ngr = len(pair)
            gc0 = TGROUPS[pair[0]][0]
            np_ = TGROUPS[pair[0]][1] * TXR if ngr == 1 else TP
            # one x tile per pair: group gi at free offset gi*WP
            xt = xpool.tile([TP, 2 * WP], BF16, tag="xt0", name=f"txt{pair[0]}")
            for gi, g in enumerate(pair):
                gg0, gnc = TGROUPS[g]
                nc.sync.dma_start(
                    _ap(xt, gi * WP, [[2 * WP, gnc * TXR], [1, WP]]),
                    bass.AP(
                        x.tensor,
                        (TH0 * C + gg0) * WP,
                        [[WP, gnc], [C * WP, TXR], [1, WP]],
                    ),
                )
            xo = xpool.tile([TP, 2 * WP], BF16, tag="xo0", name=f"txo{pair[0]}")
            nc.scalar.copy(
                _ap(xo, 0, [[2 * WP, np_], [WP, ngr], [1, 258]]),
                _ap(xt, 1, [[2 * WP, np_], [WP, ngr], [1, 258]]),
            )

            qp = TQ if ngr == 2 else TGROUPS[pair[0]][1] * TNH
            pp = np_
            ps = mmpool.tile([TQ, 2 * W], F32, tag="pst", name=f"pst{pair[0]}", bufs=1)
            for i in range(K):
                kt = ktt[i]
                pt = ppool.tile([TP, 2 * K * W], BF16, tag="pt", name=f"pt{pair[0]}_{i}")
                nc.vector.tensor_mul(
                    _ap(pt, 0, [[2 * K * W, np_], [W, ngr], [2 * W, 3], [1, W]]),
                    _ap(xt, 0, [[2 * WP, np_], [WP, ngr], [2, 3], [1, W]]),
                    _ap(kt, 0, [[K * W, np_], [0, ngr], [2 * W, 3], [1, W]]),
                )
                nc.vector.tensor_mul(
                    _ap(pt, 6 * W, [[2 * K * W, np_], [W, ngr], [2 * W, 2], [1, W]]),
                    _ap(xo, 0, [[2 * WP, np_], [WP, ngr], [2, 2], [1, W]]),
                    _ap(kt, W, [[K * W, np_], [0, ngr], [2 * W, 2], [1, W]]),
                )

                for j in range(K):
                    slot = SLOT_OF_J[j]
                    mv = _ap(
                        pt, slot * 2 * W, [[2 * K * W, pp], [1, ngr * W]]
                    )
                    nc.tensor.matmul(
                        ps[0:qp, 0 : ngr * W],
                        sttile[0:pp, i * TQ : i * TQ + qp],
                        mv,
                        start=(i == 0 and j == 0),
                        stop=(i == K - 1 and j == K - 1),
                    )

            ot = opool.tile([TQ, 2 * W], BF16, tag="ott", name=f"tot{pair[0]}")
            nc.scalar.copy(ot[0:qp, 0 : ngr * W], ps[0:qp, 0 : ngr * W])
            for gi, g in enumerate(pair):
                gc0, gnc = TGROUPS[g]
                for c in range(gnc):
                    nc.sync.dma_start(
                        out[TH0 : TH0 + TNH, gc0 + c, :],
                        ot[c * TNH : (c + 1) * TNH, gi * W : (gi + 1) * W],
                    )

        tail_pairs = [(0, 1), (2, 3), (4, 5), (6,)]
        tail_sched = {(0, 56): 0, (0, 64): 1, (124, 16): 2, (124, 40): 3}

        # ---------------- main blocks ----------------
        for h0 in BLOCKS:
            xt_pre = {}
            for c0 in (0, CG, 2 * CG):
                xt = xpool.tile([XR, CG * WP], BF16, tag="xe", name=f"xpre{h0}_{c0}")
                for ci in range(0, CG, 2):
                    nc.sync.dma_start(
                        _ap(xt, ci * WP, [[CG * WP, XR], [WP, 2], [1, WP]]),
                        x[h0 : h0 + XR, c0 + ci : c0 + ci + 2, :],
                    )
                xt_pre[c0] = xt
            # k tiles per tap-row i: [128, 5j*256]; row p holds k row h0+p-i
            ktiles = []
            for i in range(K):
                kt = kpool.tile([XR, K * W], BF16, tag=f"k{i}")
                lo = h0 - i
                klo, khi = max(0, lo), min(H, lo + XR)
                if klo > lo or khi < lo + XR:
                    nc.gpsimd.memset(kt[:], 0.0)
                for j in range(K):
                    nc.gpsimd.dma_start(
                        kt[klo - lo : khi - lo, j * W : (j + 1) * W],
                        k[i, klo:khi, j, :],
                    )
                ktiles.append(kt)

            if not sdone:
                nc.sync.dma_start(stile[:], s[:])
                nc.sync.dma_start(sttile[:], st[:])
                sdone.append(1)

            for c0 in range(0, C, CG):
                if h0 == BLOCKS[0] and CG <= c0 < (1 + K) * CG:
                    emit_tail_prep(c0 // CG - 1)
                if c0 in xt_pre:
                    xt = xt_pre[c0]
                else:
                    xt = xpool.tile([XR, CG * WP], BF16, tag="xe")
                    for ci in range(0, CG, 2):
                        nc.sync.dma_start(
                            _ap(xt, ci * WP, [[CG * WP, XR], [WP, 2], [1, WP]]),
                            x[h0 : h0 + XR, c0 + ci : c0 + ci + 2, :],
                        )
                # one-column-left-shifted copy for odd-j taps
                xo = xpool.tile([XR, CG * WP], BF16, tag="xo")
                if h0 == 0 and c0 == 0:
                    for cb in (0, CG // 2):
                        nc.scalar.copy(
                            _ap(xo, cb * WP, [[CG * WP, XR], [WP, CG // 2], [1, 258]]),
                            _ap(xt, cb * WP + 1, [[CG * WP, XR], [WP, CG // 2], [1, 258]]),
                        )
                else:
                    nc.scalar.copy(
                        _ap(xo, 0, [[CG * WP, XR], [WP, CG], [1, 258]]),
                        _ap(xt, 1, [[CG * WP, XR], [WP, CG], [1, 258]]),
                    )

                # psum accumulators, one per channel pair, live across all i
                pss = [
                    mmpool.tile(
                        [NH, 2 * W], F32, tag=f"ps{cp}", name=f"ps{cp}",
                        bufs=(2 if cp < 3 else 1),
                    )
                    for cp in range(CG // 2)
                ]
                # first group: split ops by channel halves so compute starts
                # as soon as the first half of the x tile has landed
                csplit = [(0, CG)] if not (h0 == 0 and c0 == 0) else [
                    (0, CG // 2), (CG // 2, CG // 2)
                ]
                for i in range(K):
                    kt = ktiles[i]
                    # products [128, (5slot)(8c)(256w)] bf16, fresh per i
                    p = ppool.tile([XR, CG * K * W], BF16, tag="p")
                    for cb, cn in csplit:
                        # even j {0,2,4} -> slots 0..2; layout (slot, c, w)
                        nc.vector.tensor_mul(
                            _ap(p, cb * W, [[CG * K * W, XR], [W, cn], [CG * W, 3], [1, W]]),
                            _ap(xt, cb * WP, [[CG * WP, XR], [WP, cn], [2, 3], [1, W]]),
                            _ap(kt, 0, [[K * W, XR], [0, cn], [2 * W, 3], [1, W]]),
                        )
                        # odd j {1,3} -> slots 3..4
                        nc.vector.tensor_mul(
                            _ap(p, 3 * CG * W + cb * W, [[CG * K * W, XR], [W, cn], [CG * W, 2], [1, W]]),
                            _ap(xo, cb * WP, [[CG * WP, XR], [WP, cn], [2, 2], [1, W]]),
                            _ap(kt, W, [[K * W, XR], [0, cn], [2 * W, 2], [1, W]]),
                        )

                    for cp in range(CG // 2):
                        for j in range(K):
                            slot = SLOT_OF_J[j]
                            mv = _ap(
                                p,
                                slot * CG * W + cp * 2 * W,
                                [[CG * K * W, XR], [1, 2 * W]],
                            )
                            nc.tensor.matmul(
                                pss[cp][:],
                                stile[:, i * XR : i * XR + NH],
                                mv,
                                start=(i == 0 and j == 0),
                                stop=(i == K - 1 and j == K - 1),
                            )

                # psum -> sbuf -> dram (per channel pair)
                for cp in range(CG // 2):
                    ot = opool.tile([NH, 2 * W], BF16)
                    nc.scalar.copy(ot[:], pss[cp][:])
                    nc.sync.dma_start(
                        out[h0 : h0 + NH, c0 + 2 * cp : c0 + 2 * cp + 2, :],
                        ot[:].rearrange("p (c w) -> p c w", c=2),
                    )
                tp = tail_sched.get((h0, c0 + CG))
                if tp is not None:
                    emit_tail_pair(tail_pairs[tp])

    nc.compile()
    return nc


def _get_nc():
    if "nc" not in _CACHE:
        _CACHE["nc"] = _build_nc()
    return _CACHE["nc"]


def _s_const():
    # S_i[p, h] = 1 iff p == h + i ; layout [XR, K*XR]
    s = np.zeros((K, XR, XR), dtype=np.float32)
    for i in range(K):
        s[i] = np.eye(XR, XR, -i)
    return np.ascontiguousarray(
        s.transpose(1, 0, 2).reshape(XR, K * XR)
    ).astype(BF16_NP)


def _st_const():
    # S_tail_i[(c,r), (c',q)] = 1 iff c==c' and r == q + i ; layout [TP, K*TQ]
    stm = np.zeros((K, TP, TQ), dtype=np.float32)
    for i in range(K):
        for c in range(TCG):
            for q in range(TNH):
                stm[i, c * TXR + q + i, c * TNH + q] = 1.0
    return np.ascontiguousarray(
        stm.transpose(1, 0, 2).reshape(TP, K * TQ)
    ).astype(BF16_NP)


def run(x, kernel, trace=False):
    """x: [8,64,260,260] f32, kernel: [8,25,256,256] f32 -> ([8,64,256,256], exec_ns)."""
    nc = _get_nc()
    xb = np.asarray(x).astype(BF16_NP)
    kb = np.asarray(kernel).astype(BF16_NP)
    sc, stc = _s_const(), _st_const()
    in_maps = []
    for b in range(8):
        xr = np.ascontiguousarray(xb[b].transpose(1, 0, 2))  # [h, c, w]
        kr = np.ascontiguousarray(
            kb[b].reshape(K, K, H, W).transpose(0, 2, 1, 3)
        )  # [i, h, j, w]
        in_maps.append({"x": xr, "k": kr, "s": sc, "st": stc})
    res = run_bass_kernel_spmd(nc, in_maps, core_ids=list(range(8)), trace=trace)
    outs = []
    for b in range(8):
        o = np.asarray(res.results[b]["out"], dtype=np.float32)  # [h, c, w]
        outs.append(o.transpose(1, 0, 2))
    return np.ascontiguousarray(np.stack(outs, axis=0)), res.exec_time_ns


def kernel(**inputs):
    out, _ = run(inputs["x"], inputs["kernel"], trace=False)
    return out
